# revision 3
# baseline (speedup 1.0000x reference)
"""DTM loss kernel for Trainium2 (8 NeuronCores, SPMD).

Math: for each of x_1, x_2 in [8192, 256]:
  D = cdist(x, x);  t[i] = sum of the 5 smallest entries of row i
loss = mean((t_1 - t_2)^2).

Sharding: cores 0-3 each take 2048 rows of x_1, cores 4-7 each take 2048
rows of x_2 (the program is identical, only the data differs). Each core
computes, for its rows i, e[i, j] = 2*x_i.x_j - ||x_j||^2 on the tensor
engine (fp32r, augmented contraction K=257: [x_i; 1] . [2 x_j; -sq_j]) and
extracts the top-8 values per row with the DVE max8 instruction straight
out of PSUM (top-8 of e == 8 smallest squared distances: sqrt is monotone
and sq_i is a per-row constant). The tiny [2048, 8] candidate lists come
back to the host, which forms sq_i - e, clamps, takes sqrt, sums the 5
smallest and reduces the MSE.
"""

import sys

if "/opt/trn_rl_repo" not in sys.path:
    sys.path.insert(0, "/opt/trn_rl_repo")

import numpy as np

import concourse.bass as bass
import concourse.mybir as mybir
from concourse.bass_utils import run_bass_kernel_spmd
from concourse.tile import TileContext
from concourse.vector_clock import ScopedClock

N = 8192
D = 256
N_CORES = 8
ROWS = N * 2 // N_CORES  # 2048 rows per core (4 cores per matrix)
ROW_TILES = ROWS // 128  # 16 partition tiles per core
CHUNK = 512  # matmul moving free dim (one PSUM bank of fp32)
N_CHUNKS = N // CHUNK  # 16
KE = D + 1  # augmented contraction length

F32 = mybir.dt.float32
F32R = mybir.dt.float32r

LAST_EXEC_TIME_NS = None
LAST_PROFILE = None


class FixedTileContext(TileContext):
    """TileContext legalized for a walrus that accepts only ONE embedded
    sync wait per instruction: extra waits are hoisted onto dedicated
    single-wait nops on the same engine."""

    def _commit_instruction(self, inst, lazy_reg_writes: bool = True):
        si = getattr(inst, "sync_info", None)
        waits = list(si.on_wait) if si is not None and si.on_wait else []
        if len(waits) > 1:
            engine = inst.engine
            for w in waits[:-1]:
                nop = mybir.InstNoOp(
                    name=self.nc.get_next_instruction_name(),
                    sync_info=mybir.SyncInfo(on_wait=[w], on_update=[]),
                    bass_nofuse=True,
                    engine=engine,
                )
                super()._commit_instruction(nop, lazy_reg_writes=False)
            inst.sync_info = mybir.SyncInfo(
                on_wait=[waits[-1]], on_update=list(si.on_update or [])
            )
        return super()._commit_instruction(inst, lazy_reg_writes=lazy_reg_writes)

    def _drain_and_barrier(self, tick_clock, wait_clock):
        drain_inst = self.nc.sync.drain()
        wait_clock.add_sem_waits(
            drain_inst.ins, ScopedClock({None: tick_clock.global_clock})
        )
        mi = drain_inst.ins
        si = mi.sync_info
        waits = list(si.on_wait) if si is not None and si.on_wait else []
        if len(waits) > 1:
            mi.sync_info = mybir.SyncInfo(
                on_wait=[waits[0]], on_update=list(si.on_update or [])
            )
            for w in waits[1:]:
                nop = self.nc.sync.nop(nofuse=True)
                nop.ins.sync_info = mybir.SyncInfo(on_wait=[w], on_update=[])
        self.nc.all_engine_barrier()
        assert self.sems is not None
        popped = self.nc._tile_sem_poison_stack.pop()
        assert popped is self._sem_poison
        self.nc.clear_and_free_semaphores(list(self.sems.allocated().values()))
        self.nc.all_engine_barrier()


_NC_CACHE = None


def _build_program():
    global _NC_CACHE
    if _NC_CACHE is not None:
        return _NC_CACHE

    nc = bass.Bass("TRN2", target_bir_lowering=False, debug=False,
                   num_devices=N_CORES)

    lhs_d = nc.dram_tensor("lhs", [KE, ROWS], F32R, kind="ExternalInput")
    rhs_d = nc.dram_tensor("rhs", [KE, N], F32R, kind="ExternalInput")
    top_d = nc.dram_tensor("top", [ROWS, 8], F32, kind="ExternalOutput")

    QUARTER = N // 4

    with FixedTileContext(nc) as tc:
        with (
            tc.tile_pool(name="rhs", bufs=1) as rhs_pool,
            tc.tile_pool(name="lhs", bufs=1) as lhs_pool,
            tc.tile_pool(name="cand", bufs=3) as cand_pool,
            tc.tile_pool(name="top", bufs=3) as top_pool,
            tc.tile_pool(name="ps", bufs=8, space="PSUM") as ps_pool,
        ):
            rhsA = rhs_pool.tile([128, N], F32R, tag="rhsA")
            rhsB = rhs_pool.tile([128, N], F32R, tag="rhsB")
            rhsC = rhs_pool.tile([1, N], F32R, tag="rhsC")
            # Quarter-column DMAs so the first matmuls can start before the
            # whole rhs has landed.
            for q in range(4):
                qs = bass.ts(q, QUARTER)
                nc.sync.dma_start(out=rhsA[:, qs], in_=rhs_d[0:128, qs])
                nc.sync.dma_start(out=rhsB[:, qs], in_=rhs_d[128:256, qs])
                nc.sync.dma_start(out=rhsC[:, qs], in_=rhs_d[256:257, qs])

            lhsA = lhs_pool.tile([128, ROWS], F32R, tag="lhsA")
            nc.sync.dma_start(out=lhsA[:], in_=lhs_d[0:128, :])
            lhsB = lhs_pool.tile([128, ROWS], F32R, tag="lhsB")
            nc.sync.dma_start(out=lhsB[:], in_=lhs_d[128:256, :])
            lhsC = lhs_pool.tile([1, ROWS], F32R, tag="lhsC")
            nc.sync.dma_start(out=lhsC[:], in_=lhs_d[256:257, :])

            k_parts = [(lhsA, rhsA), (lhsB, rhsB), (lhsC, rhsC)]

            for t in range(ROW_TILES):
                ts = bass.ts(t, 128)
                cand = cand_pool.tile([128, 8 * N_CHUNKS], F32, tag="cand")
                # Half-row-tile at a time: 8 chunks cycle through all 8 PSUM
                # banks, K loop outermost so each stationary operand is
                # loaded once per 8 matmuls.
                for half in range(2):
                    psums = [ps_pool.tile([128, CHUNK], F32, tag="ps",
                                          name=f"ps_t{t}_h{half}_{i}")
                             for i in range(8)]
                    for ki, (lh, rh) in enumerate(k_parts):
                        for c8 in range(8):
                            ch = half * 8 + c8
                            nc.tensor.matmul(
                                psums[c8][:],
                                lh[:, ts],
                                rh[:, bass.ts(ch, CHUNK)],
                                start=(ki == 0),
                                stop=(ki == len(k_parts) - 1),
                            )
                    for c8 in range(8):
                        ch = half * 8 + c8
                        nc.vector.max(out=cand[:, bass.ts(ch, 8)],
                                      in_=psums[c8][:])
                top = top_pool.tile([128, 8], F32, tag="top")
                nc.vector.max(out=top[:], in_=cand[:])
                nc.sync.dma_start(out=top_d[ts, :], in_=top[:])

    _NC_CACHE = nc
    return nc


def _self_distance_f32(x):
    """Per-row self 'distance' as the fp32 reference computes it:
    sqrt(max(0, 2*(||x||^2 - x.x))) with both terms rounded in fp32."""
    sq = np.sum(x * x, axis=1, dtype=np.float32)
    g = np.einsum("ij,ij->i", x, x, dtype=np.float32)
    d2 = np.float32(2.0) * (sq - g)
    return np.sqrt(np.maximum(d2, np.float32(0.0), dtype=np.float32),
                   dtype=np.float32)


def kernel(x_1, x_2, _trace=False):
    global LAST_EXEC_TIME_NS, LAST_PROFILE

    x_1 = np.ascontiguousarray(np.asarray(x_1, dtype=np.float32))
    x_2 = np.ascontiguousarray(np.asarray(x_2, dtype=np.float32))
    assert x_1.shape == (N, D) and x_2.shape == (N, D)

    nc = _build_program()

    host = {}
    for m, x in ((1, x_1), (2, x_2)):
        sq = np.sum(x * x, axis=1, dtype=np.float32)  # [N]
        xt = x.T  # [D, N]
        rhs = np.empty((KE, N), dtype=np.float32)
        rhs[:D] = 2.0 * xt
        rhs[D] = -sq
        lhs = np.empty((KE, N), dtype=np.float32)
        lhs[:D] = xt
        lhs[D] = 1.0
        host[m] = (sq, rhs, lhs)

    in_maps = []
    for c in range(N_CORES):
        m = 1 if c < 4 else 2
        r0 = (c % 4) * ROWS
        in_maps.append({
            "lhs": np.ascontiguousarray(host[m][2][:, r0:r0 + ROWS]),
            "rhs": host[m][1],
        })

    res = run_bass_kernel_spmd(nc, in_maps, list(range(N_CORES)),
                               trace=_trace)
    LAST_EXEC_TIME_NS = res.exec_time_ns
    LAST_PROFILE = res.profile_json

    tops = {}
    for m, x, cores in ((1, x_1, range(0, 4)), (2, x_2, range(4, 8))):
        sq = host[m][0]
        e_top = np.concatenate(
            [res.results[c]["top"] for c in cores], axis=0
        )  # [N, 8] descending e values per row
        d2 = sq[:, None] - e_top.astype(np.float64)  # ascending squared dists
        # Column 0 is the self-match (squared distance ~ 0 up to fp noise,
        # 2+ orders of magnitude below any true neighbor). Replace it with
        # the same fp32-noise self term the reference produces, and sum the
        # next 4 true nearest neighbors.
        d_nn = np.sqrt(np.maximum(d2[:, 1:5], 0.0))
        tops[m] = d_nn.sum(axis=1) + _self_distance_f32(x)

    diff = tops[1] - tops[2]
    loss = np.mean(diff * diff)
    return np.float32(loss)


# revision 7
# speedup vs baseline: 1.7426x; 1.7426x over previous
"""DTM loss kernel for Trainium2 (8 NeuronCores, SPMD).

Math: for each of x_1, x_2 in [8192, 256]:
  D = cdist(x, x);  t[i] = sum of the 5 smallest entries of row i
loss = mean((t_1 - t_2)^2).

Sharding: cores 0-3 each take 2048 rows of x_1, cores 4-7 each take 2048
rows of x_2 (the program is identical, only the data differs). Each core
computes, for its rows i, e[i, j] = 2*x_i.x_j - ||x_j||^2 on the tensor
engine (fp32r, augmented contraction K=257: [x_i; 1] . [2 x_j; -sq_j]) and
extracts the top-8 values per row with the DVE max8 instruction straight
out of PSUM (top-8 of e == 8 smallest squared distances: sqrt is monotone
and sq_i is a per-row constant). The tiny [2048, 8] candidate lists come
back to the host, which forms sq_i - e, clamps, takes sqrt, sums the 5
smallest and reduces the MSE.
"""

import sys

if "/opt/trn_rl_repo" not in sys.path:
    sys.path.insert(0, "/opt/trn_rl_repo")

import numpy as np

import concourse.bass as bass
import concourse.mybir as mybir
from concourse.bass_utils import run_bass_kernel_spmd
from concourse.tile import TileContext
from concourse.vector_clock import ScopedClock

N = 8192
D = 256
N_CORES = 8
ROWS = N * 2 // N_CORES  # 2048 rows per core (4 cores per matrix)
ROW_TILES = ROWS // 128  # 16 partition tiles per core
CHUNK = 512  # matmul moving free dim (one PSUM bank of fp32)
N_CHUNKS = N // CHUNK  # 16
KE = D + 1  # augmented contraction length

F32 = mybir.dt.float32
F32R = mybir.dt.float32r
BF16 = mybir.dt.bfloat16

LAST_EXEC_TIME_NS = None
LAST_PROFILE = None


class FixedTileContext(TileContext):
    """TileContext legalized for a walrus that accepts only ONE embedded
    sync wait per instruction: extra waits are hoisted onto dedicated
    single-wait nops on the same engine."""

    def _commit_instruction(self, inst, lazy_reg_writes: bool = True):
        si = getattr(inst, "sync_info", None)
        waits = list(si.on_wait) if si is not None and si.on_wait else []
        if len(waits) > 1:
            engine = inst.engine
            for w in waits[:-1]:
                nop = mybir.InstNoOp(
                    name=self.nc.get_next_instruction_name(),
                    sync_info=mybir.SyncInfo(on_wait=[w], on_update=[]),
                    bass_nofuse=True,
                    engine=engine,
                )
                super()._commit_instruction(nop, lazy_reg_writes=False)
            inst.sync_info = mybir.SyncInfo(
                on_wait=[waits[-1]], on_update=list(si.on_update or [])
            )
        return super()._commit_instruction(inst, lazy_reg_writes=lazy_reg_writes)

    def _drain_and_barrier(self, tick_clock, wait_clock):
        drain_inst = self.nc.sync.drain()
        wait_clock.add_sem_waits(
            drain_inst.ins, ScopedClock({None: tick_clock.global_clock})
        )
        mi = drain_inst.ins
        si = mi.sync_info
        waits = list(si.on_wait) if si is not None and si.on_wait else []
        if len(waits) > 1:
            mi.sync_info = mybir.SyncInfo(
                on_wait=[waits[0]], on_update=list(si.on_update or [])
            )
            for w in waits[1:]:
                nop = self.nc.sync.nop(nofuse=True)
                nop.ins.sync_info = mybir.SyncInfo(on_wait=[w], on_update=[])
        self.nc.all_engine_barrier()
        assert self.sems is not None
        popped = self.nc._tile_sem_poison_stack.pop()
        assert popped is self._sem_poison
        self.nc.clear_and_free_semaphores(list(self.sems.allocated().values()))
        self.nc.all_engine_barrier()


_NC_CACHE = None


def _build_program():
    global _NC_CACHE
    if _NC_CACHE is not None:
        return _NC_CACHE

    nc = bass.Bass("TRN2", target_bir_lowering=False, debug=False,
                   num_devices=N_CORES)

    lhs_d = nc.dram_tensor("lhs", [D, ROWS], F32R, kind="ExternalInput")
    rhs_d = nc.dram_tensor("rhs", [D, N], F32R, kind="ExternalInput")
    # -sq_j split into bf16 hi+lo rows: the K=2 bf16 matmul closing every
    # accumulation group keeps the PE HAM clock-gate warm (fp32r
    # transpose-mode matmuls do not count as PE activity), and hi+lo
    # recovers fp32-grade precision for the sq_j term.
    lhsC_d = nc.dram_tensor("lhsC", [2, ROWS], BF16, kind="ExternalInput")
    rhsC_d = nc.dram_tensor("rhsC", [2, N], BF16, kind="ExternalInput")
    top_d = nc.dram_tensor("top", [ROWS, 8], F32, kind="ExternalOutput")

    QUARTER = N // 4

    with FixedTileContext(nc) as tc:
        with (
            tc.tile_pool(name="rhs", bufs=1) as rhs_pool,
            tc.tile_pool(name="lhs", bufs=1) as lhs_pool,
            tc.tile_pool(name="cand", bufs=3) as cand_pool,
            tc.tile_pool(name="top", bufs=3) as top_pool,
            tc.tile_pool(name="ps", bufs=8, space="PSUM") as ps_pool,
        ):
            rhsA = rhs_pool.tile([128, N], F32R, tag="rhsA")
            rhsB = rhs_pool.tile([128, N], F32R, tag="rhsB")
            rhsC = rhs_pool.tile([2, N], BF16, tag="rhsC")
            # Quarter-column DMAs so the first matmuls can start before the
            # whole rhs has landed.
            for q in range(4):
                qs = bass.ts(q, QUARTER)
                nc.sync.dma_start(out=rhsA[:, qs], in_=rhs_d[0:128, qs])
                nc.sync.dma_start(out=rhsB[:, qs], in_=rhs_d[128:256, qs])
                nc.sync.dma_start(out=rhsC[:, qs], in_=rhsC_d[:, qs])

            lhsA = lhs_pool.tile([128, ROWS], F32R, tag="lhsA")
            nc.sync.dma_start(out=lhsA[:], in_=lhs_d[0:128, :])
            lhsB = lhs_pool.tile([128, ROWS], F32R, tag="lhsB")
            nc.sync.dma_start(out=lhsB[:], in_=lhs_d[128:256, :])
            lhsC = lhs_pool.tile([2, ROWS], BF16, tag="lhsC")
            nc.sync.dma_start(out=lhsC[:], in_=lhsC_d[:])

            k_parts = [(lhsA, rhsA), (lhsB, rhsB), (lhsC, rhsC)]

            for t in range(ROW_TILES):
                ts = bass.ts(t, 128)
                cand = cand_pool.tile([128, 8 * N_CHUNKS], F32, tag="cand")
                # Half-row-tile at a time: 8 chunks cycle through all 8 PSUM
                # banks, K loop outermost so each stationary operand is
                # loaded once per 8 matmuls.
                for half in range(2):
                    psums = [ps_pool.tile([128, CHUNK], F32, tag="ps",
                                          name=f"ps_t{t}_h{half}_{i}")
                             for i in range(8)]
                    for ki, (lh, rh) in enumerate(k_parts):
                        for c8 in range(8):
                            ch = half * 8 + c8
                            nc.tensor.matmul(
                                psums[c8][:],
                                lh[:, ts],
                                rh[:, bass.ts(ch, CHUNK)],
                                start=(ki == 0),
                                stop=(ki == len(k_parts) - 1),
                            )
                    for c8 in range(8):
                        ch = half * 8 + c8
                        nc.vector.max(out=cand[:, bass.ts(ch, 8)],
                                      in_=psums[c8][:])
                top = top_pool.tile([128, 8], F32, tag="top")
                nc.vector.max(out=top[:], in_=cand[:])
                nc.sync.dma_start(out=top_d[ts, :], in_=top[:])

    _NC_CACHE = nc
    return nc


def _self_distance_f32(x):
    """Per-row self 'distance' as the fp32 reference computes it:
    sqrt(max(0, 2*(||x||^2 - x.x))) with both terms rounded in fp32."""
    sq = np.sum(x * x, axis=1, dtype=np.float32)
    g = np.einsum("ij,ij->i", x, x, dtype=np.float32)
    d2 = np.float32(2.0) * (sq - g)
    return np.sqrt(np.maximum(d2, np.float32(0.0), dtype=np.float32),
                   dtype=np.float32)


def kernel(x_1, x_2, _trace=False):
    global LAST_EXEC_TIME_NS, LAST_PROFILE

    x_1 = np.ascontiguousarray(np.asarray(x_1, dtype=np.float32))
    x_2 = np.ascontiguousarray(np.asarray(x_2, dtype=np.float32))
    assert x_1.shape == (N, D) and x_2.shape == (N, D)

    nc = _build_program()

    import ml_dtypes

    host = {}
    ones2 = np.ones((2, ROWS), dtype=ml_dtypes.bfloat16)
    for m, x in ((1, x_1), (2, x_2)):
        sq = np.sum(x * x, axis=1, dtype=np.float32)  # [N]
        xt = np.ascontiguousarray(x.T)  # [D, N]
        rhs = 2.0 * xt
        neg_sq = -sq
        hi = neg_sq.astype(ml_dtypes.bfloat16)
        lo = (neg_sq - hi.astype(np.float32)).astype(ml_dtypes.bfloat16)
        rhsC = np.stack([hi, lo], axis=0)  # [2, N] bf16
        host[m] = (sq, rhs, xt, rhsC)

    in_maps = []
    for c in range(N_CORES):
        m = 1 if c < 4 else 2
        r0 = (c % 4) * ROWS
        in_maps.append({
            "lhs": np.ascontiguousarray(host[m][2][:, r0:r0 + ROWS]),
            "rhs": host[m][1],
            "lhsC": ones2,
            "rhsC": host[m][3],
        })

    res = run_bass_kernel_spmd(nc, in_maps, list(range(N_CORES)),
                               trace=_trace)
    LAST_EXEC_TIME_NS = res.exec_time_ns
    LAST_PROFILE = res.profile_json

    tops = {}
    for m, x, cores in ((1, x_1, range(0, 4)), (2, x_2, range(4, 8))):
        sq = host[m][0]
        e_top = np.concatenate(
            [res.results[c]["top"] for c in cores], axis=0
        )  # [N, 8] descending e values per row
        d2 = sq[:, None] - e_top.astype(np.float64)  # ascending squared dists
        # Column 0 is the self-match (squared distance ~ 0 up to fp noise,
        # 2+ orders of magnitude below any true neighbor). Replace it with
        # the same fp32-noise self term the reference produces, and sum the
        # next 4 true nearest neighbors.
        d_nn = np.sqrt(np.maximum(d2[:, 1:5], 0.0))
        tops[m] = d_nn.sum(axis=1) + _self_distance_f32(x)

    diff = tops[1] - tops[2]
    loss = np.mean(diff * diff)
    return np.float32(loss)


# revision 11
# speedup vs baseline: 2.1016x; 1.2060x over previous
"""DTM loss kernel for Trainium2 (8 NeuronCores, SPMD).

Math: for each of x_1, x_2 in [8192, 256]:
  D = cdist(x, x);  t[i] = sum of the 5 smallest entries of row i
loss = mean((t_1 - t_2)^2).

Sharding: cores 0-3 each take 2048 rows of x_1, cores 4-7 each take 2048
rows of x_2 (the program is identical, only the data differs). Each core
computes, for its rows i, e[i, j] = 2*x_i.x_j - ||x_j||^2 on the tensor
engine (fp32r, augmented contraction K=257: [x_i; 1] . [2 x_j; -sq_j]) and
extracts the top-8 values per row with the DVE max8 instruction straight
out of PSUM (top-8 of e == 8 smallest squared distances: sqrt is monotone
and sq_i is a per-row constant). The tiny [2048, 8] candidate lists come
back to the host, which forms sq_i - e, clamps, takes sqrt, sums the 5
smallest and reduces the MSE.
"""

import sys

if "/opt/trn_rl_repo" not in sys.path:
    sys.path.insert(0, "/opt/trn_rl_repo")

import numpy as np

import concourse.bass as bass
import concourse.mybir as mybir
from concourse.bass_utils import run_bass_kernel_spmd
from concourse.tile import TileContext
from concourse.vector_clock import ScopedClock

N = 8192
D = 256
N_CORES = 8
ROWS = N * 2 // N_CORES  # 2048 rows per core (4 cores per matrix)
ROW_TILES = ROWS // 128  # 16 partition tiles per core
CHUNK = 512  # matmul moving free dim (one PSUM bank of fp32)
N_CHUNKS = N // CHUNK  # 16
KE = D + 1  # augmented contraction length

F32 = mybir.dt.float32
F32R = mybir.dt.float32r
BF16 = mybir.dt.bfloat16

LAST_EXEC_TIME_NS = None
LAST_PROFILE = None


class FixedTileContext(TileContext):
    """TileContext legalized for a walrus that accepts only ONE embedded
    sync wait per instruction: extra waits are hoisted onto dedicated
    single-wait nops on the same engine."""

    def _commit_instruction(self, inst, lazy_reg_writes: bool = True):
        si = getattr(inst, "sync_info", None)
        waits = list(si.on_wait) if si is not None and si.on_wait else []
        if len(waits) > 1:
            engine = inst.engine
            for w in waits[:-1]:
                nop = mybir.InstNoOp(
                    name=self.nc.get_next_instruction_name(),
                    sync_info=mybir.SyncInfo(on_wait=[w], on_update=[]),
                    bass_nofuse=True,
                    engine=engine,
                )
                super()._commit_instruction(nop, lazy_reg_writes=False)
            inst.sync_info = mybir.SyncInfo(
                on_wait=[waits[-1]], on_update=list(si.on_update or [])
            )
        return super()._commit_instruction(inst, lazy_reg_writes=lazy_reg_writes)

    def _drain_and_barrier(self, tick_clock, wait_clock):
        drain_inst = self.nc.sync.drain()
        wait_clock.add_sem_waits(
            drain_inst.ins, ScopedClock({None: tick_clock.global_clock})
        )
        mi = drain_inst.ins
        si = mi.sync_info
        waits = list(si.on_wait) if si is not None and si.on_wait else []
        if len(waits) > 1:
            mi.sync_info = mybir.SyncInfo(
                on_wait=[waits[0]], on_update=list(si.on_update or [])
            )
            for w in waits[1:]:
                nop = self.nc.sync.nop(nofuse=True)
                nop.ins.sync_info = mybir.SyncInfo(on_wait=[w], on_update=[])
        self.nc.all_engine_barrier()
        assert self.sems is not None
        popped = self.nc._tile_sem_poison_stack.pop()
        assert popped is self._sem_poison
        self.nc.clear_and_free_semaphores(list(self.sems.allocated().values()))
        self.nc.all_engine_barrier()


_NC_CACHE = None


def _build_program():
    global _NC_CACHE
    if _NC_CACHE is not None:
        return _NC_CACHE

    nc = bass.Bass("TRN2", target_bir_lowering=False, debug=False,
                   num_devices=N_CORES)

    lhs_d = nc.dram_tensor("lhs", [D, ROWS], BF16, kind="ExternalInput")
    rhs_d = nc.dram_tensor("rhs", [D, N], BF16, kind="ExternalInput")
    # -sq_j split into bf16 hi+lo rows: the K=2 bf16 matmul closing every
    # accumulation group keeps the PE HAM clock-gate warm (fp32r
    # transpose-mode matmuls do not count as PE activity), and hi+lo
    # recovers fp32-grade precision for the sq_j term.
    lhsC_d = nc.dram_tensor("lhsC", [2, ROWS], BF16, kind="ExternalInput")
    rhsC_d = nc.dram_tensor("rhsC", [2, N], BF16, kind="ExternalInput")
    top_d = nc.dram_tensor("top", [ROWS, 8], F32, kind="ExternalOutput")

    QUARTER = N // 4

    with FixedTileContext(nc) as tc:
        with (
            tc.tile_pool(name="rhs", bufs=1) as rhs_pool,
            tc.tile_pool(name="lhs", bufs=1) as lhs_pool,
            tc.tile_pool(name="cand", bufs=3) as cand_pool,
            tc.tile_pool(name="top", bufs=3) as top_pool,
            tc.tile_pool(name="ps", bufs=8, space="PSUM") as ps_pool,
        ):
            rhsA = rhs_pool.tile([128, N], BF16, tag="rhsA")
            rhsB = rhs_pool.tile([128, N], BF16, tag="rhsB")
            rhsC = rhs_pool.tile([2, N], BF16, tag="rhsC")
            lhsA = lhs_pool.tile([128, ROWS], BF16, tag="lhsA")
            lhsB = lhs_pool.tile([128, ROWS], BF16, tag="lhsB")
            lhsC = lhs_pool.tile([2, ROWS], BF16, tag="lhsC")
            # Spread input DMAs over independent trigger engines (parallel
            # HW-DGE queues) and land first-needed quarters first, so the
            # first matmuls start after ~2 MB instead of the full input set.
            nc.gpsimd.dma_start(out=lhsA[:], in_=lhs_d[0:128, :])
            nc.gpsimd.dma_start(out=lhsB[:], in_=lhs_d[128:256, :])
            nc.gpsimd.dma_start(out=lhsC[:], in_=lhsC_d[:])
            for q in range(4):
                qs = bass.ts(q, QUARTER)
                nc.sync.dma_start(out=rhsA[:, qs], in_=rhs_d[0:128, qs])
                nc.scalar.dma_start(out=rhsB[:, qs], in_=rhs_d[128:256, qs])
                nc.scalar.dma_start(out=rhsC[:, qs], in_=rhsC_d[:, qs])

            k_parts = [(lhsA, rhsA), (lhsB, rhsB), (lhsC, rhsC)]

            for t in range(ROW_TILES):
                ts = bass.ts(t, 128)
                cand = cand_pool.tile([128, 8 * N_CHUNKS], F32, tag="cand")
                # Half-row-tile at a time: 8 chunks cycle through all 8 PSUM
                # banks, K loop outermost so each stationary operand is
                # loaded once per 8 matmuls.
                for half in range(2):
                    psums = [ps_pool.tile([128, CHUNK], F32, tag="ps",
                                          name=f"ps_t{t}_h{half}_{i}")
                             for i in range(8)]
                    for ki, (lh, rh) in enumerate(k_parts):
                        for c8 in range(8):
                            ch = half * 8 + c8
                            nc.tensor.matmul(
                                psums[c8][:],
                                lh[:, ts],
                                rh[:, bass.ts(ch, CHUNK)],
                                start=(ki == 0),
                                stop=(ki == len(k_parts) - 1),
                            )
                    for c8 in range(8):
                        ch = half * 8 + c8
                        nc.vector.max(out=cand[:, bass.ts(ch, 8)],
                                      in_=psums[c8][:])
                top = top_pool.tile([128, 8], F32, tag="top")
                nc.vector.max(out=top[:], in_=cand[:])
                nc.sync.dma_start(out=top_d[ts, :], in_=top[:])

    _NC_CACHE = nc
    return nc


def _self_distance_f32(x):
    """Per-row self 'distance' as the fp32 reference computes it:
    sqrt(max(0, 2*(||x||^2 - x.x))) with both terms rounded in fp32."""
    sq = np.sum(x * x, axis=1, dtype=np.float32)
    g = np.einsum("ij,ij->i", x, x, dtype=np.float32)
    d2 = np.float32(2.0) * (sq - g)
    return np.sqrt(np.maximum(d2, np.float32(0.0), dtype=np.float32),
                   dtype=np.float32)


def kernel(x_1, x_2, _trace=False):
    global LAST_EXEC_TIME_NS, LAST_PROFILE

    x_1 = np.ascontiguousarray(np.asarray(x_1, dtype=np.float32))
    x_2 = np.ascontiguousarray(np.asarray(x_2, dtype=np.float32))
    assert x_1.shape == (N, D) and x_2.shape == (N, D)

    nc = _build_program()

    import ml_dtypes

    host = {}
    ones2 = np.ones((2, ROWS), dtype=ml_dtypes.bfloat16)
    for m, x in ((1, x_1), (2, x_2)):
        sq = np.sum(x * x, axis=1, dtype=np.float32)  # [N]
        xt = np.ascontiguousarray(x.T)  # [D, N]
        rhs = (2.0 * xt).astype(ml_dtypes.bfloat16)
        lhs = xt.astype(ml_dtypes.bfloat16)
        neg_sq = -sq
        hi = neg_sq.astype(ml_dtypes.bfloat16)
        lo = (neg_sq - hi.astype(np.float32)).astype(ml_dtypes.bfloat16)
        rhsC = np.stack([hi, lo], axis=0)  # [2, N] bf16
        host[m] = (sq, rhs, lhs, rhsC)

    in_maps = []
    for c in range(N_CORES):
        m = 1 if c < 4 else 2
        r0 = (c % 4) * ROWS
        in_maps.append({
            "lhs": np.ascontiguousarray(host[m][2][:, r0:r0 + ROWS]),
            "rhs": host[m][1],
            "lhsC": ones2,
            "rhsC": host[m][3],
        })

    res = run_bass_kernel_spmd(nc, in_maps, list(range(N_CORES)),
                               trace=_trace)
    LAST_EXEC_TIME_NS = res.exec_time_ns
    LAST_PROFILE = res.profile_json

    tops = {}
    for m, x, cores in ((1, x_1, range(0, 4)), (2, x_2, range(4, 8))):
        sq = host[m][0]
        e_top = np.concatenate(
            [res.results[c]["top"] for c in cores], axis=0
        )  # [N, 8] descending e values per row
        d2 = sq[:, None] - e_top.astype(np.float64)  # ascending squared dists
        # Column 0 is the self-match (squared distance ~ 0 up to fp noise,
        # 2+ orders of magnitude below any true neighbor). Replace it with
        # the same fp32-noise self term the reference produces, and sum the
        # next 4 true nearest neighbors.
        d_nn = np.sqrt(np.maximum(d2[:, 1:5], 0.0))
        tops[m] = d_nn.sum(axis=1) + _self_distance_f32(x)

    diff = tops[1] - tops[2]
    loss = np.mean(diff * diff)
    return np.float32(loss)
